# revision 11
# baseline (speedup 1.0000x reference)
"""BitLinear (ternary-quantized linear) Trainium2 kernel.

out = (x @ ternary_quantize(W).T) * mean(|W|),  alpha = 0.7

Sharding: tensor-parallel over out_features (8192 -> 8 x 1024). Every core
gets the full activation x (pre-transposed on host to [K, T] layout so all
device DMAs are contiguous) plus its own weight shard, also pre-transposed
to [K, O_shard].

weight_scale = mean(|W|) needs a global sum of |w| across the 8 shards. A
[128,1] device AllReduce works but costs ~150us per execution in this
environment (vs the ~10us documented floor), so the default is two
launches: launch 1 computes each core's partial |w| sums on device;
the host concatenates the 8 [128,1] vectors (pure data movement, no host
math) and feeds them to every core in launch 2, which sums them on device.
This also removes the weight-scan from launch 2's critical path: the
threshold is available immediately, so quantization pipelines with the
weight DMA and the tensor engine starts ~20us into the kernel.

Device kernel per core in the main launch (SPMD, identical program):
  phase 0: global mean / 0.7*mean threshold replicated on 128 partitions
           (from the gsums input; AllReduce fallback via KERNEL_CC=ar)
  phase 1: ternary-quantize the fp32 shard into {-1, 0, +1} (exact):
           k-tiles [0, F8) into fp8-e4m3 wq8, the rest into fp16 wq16;
           k-major so phase 2's first k-step is ready early, pipelined
           with the per-k-tile weight DMAs
  phase 2: tiled matmul. Per 512-token chunk, all 8 (tsub, o-half) PSUM
           banks accumulate concurrently with the k-step loop OUTERMOST:
           fp8 k-pairs via DoubleRow matmuls ([128,2,128]x[128,2,512],
           2 k-tiles per instruction), then fp16 k-tiles. The k-outer
           order means each stationary x tile is loaded once per k-step
           and reused across both o-halves, which keeps the 256-column
           DoubleRow LDWEIGHTS off the critical path (measured 1.6x vs
           the psum-group-inner-k order). x is cast fp32->fp8/fp16
           in-flight by the SWDGE DMA. PSUM is scaled by mean(|W|) on
           the scalar engine during the PSUM->SBUF copy, then DMA'd out.

Precision: weights {-1,0,+1} are exact in both fp8-e4m3 and fp16; all
error comes from quantizing x. fp16 gives rel ~1.5e-4; full fp8 gives
1.93e-2 (vs the 2e-2 gate). F8=12 of 16 k-tiles in fp8 measures
1.6785e-2, reproduced bit-identically by the CPU model (the SWDGE cast
is round-to-nearest ml_dtypes.float8_e4m3), with a 400-trial Monte
Carlo over threshold-summation-order perturbations staying under
1.88e-2.
"""

import numpy as np

import concourse.mybir as mybir
import concourse.tile as tile
from concourse import bacc, bass_isa
from concourse.bass_utils import run_bass_kernel_spmd

N_CORES = 8
B, S, IN_F, OUT_F = 4, 2048, 2048, 8192
T_FULL = B * S              # 8192 tokens
K = IN_F                    # contraction dim
OS = OUT_F // N_CORES       # 1024 out-features per core
P = 128
KT = K // P                 # 16 k-tiles
ALPHA = 0.7
N_TOTAL = float(OUT_F * K)  # 2**24, so 1/N_TOTAL is exact in fp32

C = 512                     # token chunk per x DMA
NF = 512                    # matmul moving free dim (one PSUM bank fp32)

LAST_RESULTS = None         # test harness peeks at exec_time_ns here
_PROGRAMS = {}              # compiled program cache across kernel() calls

import os as _os
SKIP = set(filter(None, _os.environ.get("KERNEL_SKIP", "").split(",")))
XDT_NAME = _os.environ.get("KERNEL_XDT", "float16")  # matmul operand dtype
SCALE_ENG = _os.environ.get("KERNEL_SCALE_ENG", "scalar")  # psum-scale engine
CC_KIND = _os.environ.get("KERNEL_CC", "2launch")  # ar | ag | 2launch
# Number of k-tiles (out of KT=16) computed in fp8-e4m3 via DoubleRow
# matmuls (2 k-tiles per instruction, ~1.44x tensor-engine throughput).
# x-quantization error grows as sqrt(F8/16) * 1.9e-2; the rest of the
# k-tiles stay fp16. Must be even.
F8 = int(_os.environ.get("KERNEL_F8", "12"))
assert F8 % 2 == 0 and 0 <= F8 <= KT
# Matmul loop order. "ocmajor": psum-group inner k (stationary x tile
# changes every matmul -> DoubleRow pays its 256-col LDWEIGHTS each MM).
# "kouter": k-step outer over all 8 psum banks of a chunk, so each
# stationary x tile is reused for both o-halves (half the LDWEIGHTS).
ORDER = _os.environ.get("KERNEL_ORDER", "kouter")


def _build_program(t_tokens=T_FULL, loops=1, n_cores=N_CORES, barrier=False):
    F32 = mybir.dt.float32
    AX = mybir.AxisListType.X
    Alu = mybir.AluOpType

    assert t_tokens % C == 0 and C % P == 0 and OS % NF == 0

    nc = bacc.Bacc(
        "TRN2", target_bir_lowering=False, debug=False, num_devices=n_cores
    )
    xT = nc.dram_tensor("xT", [K, t_tokens], F32, kind="ExternalInput").ap()
    wT = nc.dram_tensor("wT", [K, OS], F32, kind="ExternalInput").ap()
    gsums = None
    if CC_KIND == "2launch":
        gsums = nc.dram_tensor(
            "gsums", [P, N_CORES], F32, kind="ExternalInput"
        ).ap()
    out = nc.dram_tensor("out", [t_tokens, OS], F32, kind="ExternalOutput").ap()

    with tile.TileContext(nc) as tc:
        for _loop in range(loops):
            if barrier and _loop > 0:
                tc.strict_bb_all_engine_barrier()
            _build_body(tc, nc, xT, wT, out, t_tokens, n_cores, gsums)

    nc.compile()
    return nc


def _build_phase_a(loops=1, barrier=False):
    """Tiny first launch: per-core sum of |w shard| -> [128, 1] output."""
    F32 = mybir.dt.float32
    AX = mybir.AxisListType.X
    Alu = mybir.AluOpType
    nc = bacc.Bacc(
        "TRN2", target_bir_lowering=False, debug=False, num_devices=N_CORES
    )
    wT = nc.dram_tensor("wT", [K, OS], F32, kind="ExternalInput").ap()
    asum_out = nc.dram_tensor("asum", [P, 1], F32, kind="ExternalOutput").ap()
    with tile.TileContext(nc) as tc:
        for _loop in range(loops):
            if barrier and _loop > 0:
                tc.strict_bb_all_engine_barrier()
            with (
                tc.tile_pool(name="wpa", bufs=3) as wpa,
                tc.tile_pool(name="cpa", bufs=1) as cpa,
            ):
                # 8 x 1MB DMAs (2 k-tiles each): above the DMA batching knee,
                # reduction still pipelines with the loads
                wT_t = wT.rearrange("(n p) o -> p n o", p=P)
                KG = 2
                asum = cpa.tile([P, KT // KG], F32)
                for g in range(KT // KG):
                    wt = wpa.tile([P, KG, OS], F32, name="wt", tag="wt")
                    nc.sync.dma_start(wt[:], wT_t[:, g * KG : (g + 1) * KG, :])
                    nc.vector.tensor_reduce(
                        asum[:, g : g + 1], wt[:],
                        axis=mybir.AxisListType.XY, op=Alu.add,
                        apply_absolute_value=True,
                    )
                asum1 = cpa.tile([P, 1], F32)
                nc.vector.tensor_reduce(asum1[:], asum[:], axis=AX, op=Alu.add)
                nc.sync.dma_start(asum_out[:], asum1[:])
    nc.compile()
    return nc


def _build_phase_a_raw(loops=1):
    """Raw-bass phase A: no TileContext, so no ~10-15us exit butterfly.

    8 HWDGE DMAs into distinct SBUF regions (no reuse -> no WAR hazards),
    DVE abs-reduces pipelined behind them via one counting semaphore.
    """
    F32 = mybir.dt.float32
    Alu = mybir.AluOpType
    XY = mybir.AxisListType.XY
    AX = mybir.AxisListType.X
    KG = 2
    NG = KT // KG  # 8 groups
    nc = bacc.Bacc(
        "TRN2", target_bir_lowering=False, debug=False, num_devices=N_CORES
    )
    wT = nc.dram_tensor("wT", [K, OS], F32, kind="ExternalInput").ap()
    asum_out = nc.dram_tensor("asum", [P, 1], F32, kind="ExternalOutput").ap()
    wT_t = wT.rearrange("(n p) o -> p n o", p=P)

    import contextlib

    with contextlib.ExitStack() as ctx:
        wt = ctx.enter_context(nc.sbuf_tensor("wt", [P, KT, OS], F32))
        asum_sb = ctx.enter_context(nc.sbuf_tensor("asum_sb", [P, NG], F32))
        asum1_sb = ctx.enter_context(nc.sbuf_tensor("asum1_sb", [P, 1], F32))
        # one completion sem per DMA group: HWDGE DMAs complete out of
        # order, so a single counting sem cannot gate per-group reduces
        gsem = [
            ctx.enter_context(nc.semaphore(f"dma_sem{g}")) for g in range(NG)
        ]
        osem = ctx.enter_context(nc.semaphore("out_sem"))
        vec_sem = ctx.enter_context(nc.semaphore("vec_sem"))
        block = ctx.enter_context(nc.Block())

        @block.sync
        def _(sync):
            for it in range(loops):
                for g in range(NG):
                    sync.dma_start(
                        wt.ap()[:, g * KG : (g + 1) * KG, :],
                        wT_t[:, g * KG : (g + 1) * KG, :],
                    ).then_inc(gsem[g], 16)
                sync.wait_ge(vec_sem, it + 1)
                sync.dma_start(asum_out[:], asum1_sb.ap()[:]).then_inc(
                    osem, 16
                )
                # last byte of the output must land before the stream ends
                sync.wait_ge(osem, (it + 1) * 16)

        @block.vector
        def _(vector):
            for it in range(loops):
                for g in range(NG):
                    vector.wait_ge(gsem[g], (it + 1) * 16)
                    vector.tensor_reduce(
                        asum_sb.ap()[:, g : g + 1],
                        wt.ap()[:, g * KG : (g + 1) * KG, :],
                        axis=XY, op=Alu.add, apply_absolute_value=True,
                    )
                vector.tensor_reduce(
                    asum1_sb.ap()[:], asum_sb.ap()[:], axis=AX, op=Alu.add
                ).then_inc(vec_sem, 1)

    nc.compile()
    return nc


def _build_body(tc, nc, xT, wT, out, t_tokens, n_cores, gsums=None):
    F32 = mybir.dt.float32
    XDT = getattr(mybir.dt, XDT_NAME)
    AX = mybir.AxisListType.X
    Alu = mybir.AluOpType
    if True:
        with (  # noqa: SIM117

            tc.tile_pool(name="wpool", bufs=1) as wpool,
            tc.tile_pool(name="cpool", bufs=1) as cpool,
            tc.tile_pool(name="dram", bufs=1, space="DRAM") as dram,
            tc.tile_pool(name="xpool", bufs=2) as xpool,
            tc.tile_pool(name="opool", bufs=6) as opool,
            tc.tile_pool(name="psum", bufs=6, space="PSUM") as psum_pool,
        ):
            # ---- phase 0: weight shard load + global mean(|W|) ----
            # per-(o-half, k-tile) DMAs in quantization order, so the first
            # o-half's quant (which gates the first matmuls) only waits for
            # half the weight bytes
            wf = wpool.tile([P, KT, OS], F32)
            wT_t = wT.rearrange("(n p) o -> p n o", p=P)
            if ORDER == "kouter":
                wdma_iter = [
                    (oc, k) for k in range(KT) for oc in range(OS // NF)
                ]
            else:
                wdma_iter = [
                    (oc, k) for oc in range(OS // NF) for k in range(KT)
                ]
            for oc, k in wdma_iter:
                osl = slice(oc * NF, (oc + 1) * NF)
                nc.sync.dma_start(wf[:, k, osl], wT_t[:, k, osl])
            if not (CC_KIND == "2launch" and gsums is not None):
                # local |w| sums feed the collective (non-2launch modes)
                asum = cpool.tile([P, KT], F32)
                for k in range(KT):
                    nc.vector.tensor_reduce(
                        asum[:, k : k + 1], wf[:, k, :], axis=AX, op=Alu.add,
                        apply_absolute_value=True,
                    )
                asum1 = cpool.tile([P, 1], F32)
                nc.vector.tensor_reduce(asum1[:], asum[:], axis=AX, op=Alu.add)

            if CC_KIND == "2launch" and gsums is not None:
                # partial |w| sums of all 8 cores arrive as an input
                gsum8 = cpool.tile([P, N_CORES], F32)
                nc.sync.dma_start(gsum8[:], gsums[:])
                gsum = cpool.tile([P, 1], F32)
                nc.vector.tensor_reduce(gsum[:], gsum8[:], axis=AX, op=Alu.add)
            elif n_cores > 1 and "ar" not in SKIP:
                if CC_KIND == "ag":
                    cc_in = dram.tile([P, 1], F32)
                    cc_out = dram.tile([n_cores * P, 1], F32)
                    nc.sync.dma_start(cc_in[:], asum1[:])
                    nc.gpsimd.collective_compute(
                        "AllGather", Alu.bypass,
                        replica_groups=[list(range(n_cores))],
                        ins=[cc_in.opt()], outs=[cc_out.opt()],
                    )
                    gsum8 = cpool.tile([P, n_cores], F32)
                    nc.sync.dma_start(
                        gsum8[:], cc_out.rearrange("(r p) o -> p (r o)", p=P)
                    )
                    gsum = cpool.tile([P, 1], F32)
                    nc.vector.tensor_reduce(
                        gsum[:], gsum8[:], axis=AX, op=Alu.add
                    )
                else:
                    cc_in = dram.tile([P, 1], F32)
                    cc_out = dram.tile([P, 1], F32)
                    nc.sync.dma_start(cc_in[:], asum1[:])
                    nc.gpsimd.collective_compute(
                        "AllReduce", Alu.add,
                        replica_groups=[list(range(n_cores))],
                        ins=[cc_in.opt()], outs=[cc_out.opt()],
                    )
                    gsum = cpool.tile([P, 1], F32)
                    nc.sync.dma_start(gsum[:], cc_out[:])
            else:
                gsum = asum1  # single-core (TimelineSim) variant

            tot = cpool.tile([P, 1], F32)
            if "par" not in SKIP:
                nc.gpsimd.partition_all_reduce(
                    tot[:], gsum[:], channels=P, reduce_op=bass_isa.ReduceOp.add
                )
            else:
                nc.vector.tensor_copy(tot[:], gsum[:])
            # mean = tot * 2**-24 (exact); thr = 0.7 * mean; both replicated
            mean_t = cpool.tile([P, 1], F32)
            nc.vector.tensor_scalar_mul(mean_t[:], tot[:], 1.0 / N_TOTAL)
            thr_t = cpool.tile([P, 1], F32)
            nc.vector.tensor_scalar_mul(thr_t[:], mean_t[:], ALPHA)
            nthr_t = cpool.tile([P, 1], F32)
            nc.vector.tensor_scalar_mul(nthr_t[:], thr_t[:], -1.0)

            # ---- phase 1: ternary quantize -> wq (exact {-1,0,+1}) ----
            # k-tiles [0, F8) go to wq8 (fp8-e4m3, consumed by DoubleRow
            # matmuls), the rest to wq16 (fp16). oc-major so the first
            # o-half is ready in half the quant time; phase 2's first chunk
            # consumes o-half 0 first.
            F8DT = mybir.dt.float8e4
            KT16 = KT - F8
            wq8 = wpool.tile([P, F8, OS], F8DT, name="wq8") if F8 else None
            wq16 = (
                wpool.tile([P, KT16, OS], XDT, name="wq16") if KT16 else None
            )

            def _wq_slot(k, osl):
                if k < F8:
                    return wq8[:, k, osl]
                return wq16[:, k - F8, osl]

            # quant iteration order matches what phase 2 consumes first:
            # ocmajor's first psum group needs all k of o-half 0; kouter's
            # first k-step needs k-tiles 0..1 across the full o range.
            if ORDER == "kouter":
                quant_iter = [
                    (oc, k) for k in range(KT) for oc in range(OS // NF)
                ]
            else:
                quant_iter = [
                    (oc, k) for oc in range(OS // NF) for k in range(KT)
                ]
            if "quant" in SKIP:
                if F8:
                    nc.vector.memset(wq8[:], 1.0)
                if KT16:
                    nc.vector.memset(wq16[:], 1.0)
            else:
                for oc, k in quant_iter:
                    osl = slice(oc * NF, (oc + 1) * NF)
                    neg = wpool.tile([P, NF], XDT, tag="negtmp")
                    # neg = (w <= -thr) in {0,1}
                    nc.vector.tensor_scalar(
                        neg[:], wf[:, k, osl], nthr_t[:], None, op0=Alu.is_le
                    )
                    # wq = (w >= thr) - neg  in {-1, 0, 1}
                    nc.vector.scalar_tensor_tensor(
                        _wq_slot(k, osl), wf[:, k, osl], thr_t[:], neg[:],
                        op0=Alu.is_ge, op1=Alu.subtract,
                    )

            # ---- phase 2: matmul sweep over tokens (oc-major per chunk so
            # the first chunk only waits on the o-half-0 quantization) ----
            xT_t = xT.rearrange("(n p) t -> p n t", p=P)
            n_chunks = t_tokens // C
            n_mm = F8 // 2 + KT16
            for tch in range(n_chunks):
                tsl_d = slice(tch * C, (tch + 1) * C)
                xb8 = xb16 = None
                if F8:
                    xb8 = xpool.tile([P, F8, C], F8DT, name="xb8", tag="xb8")
                    # SWDGE DMA with in-flight fp32 -> fp8 cast
                    nc.gpsimd.dma_start(xb8[:], xT_t[:, :F8, tsl_d])
                if KT16:
                    xb16 = xpool.tile([P, KT16, C], XDT, name="xb16", tag="xb16")
                    # SWDGE DMA with in-flight fp32 -> fp16 cast
                    nc.gpsimd.dma_start(xb16[:], xT_t[:, F8:, tsl_d])
                if ORDER == "kouter":
                    # k-step outer: all 8 (tsub, oc) psum banks accumulate
                    # concurrently, so each stationary x tile is loaded once
                    # per k-step and reused for both o-halves.
                    pos = {}
                    for tsub in range(C // P):
                        for oc in range(OS // NF):
                            pos[tsub, oc] = psum_pool.tile(
                                [P, NF], F32,
                                name="po", tag=f"po{tsub}_{oc}", bufs=1,
                            )
                    if "mm" not in SKIP:
                        steps = [("f8", kp) for kp in range(F8 // 2)]
                        steps += [("f16", k) for k in range(KT16)]
                        for si, (kind, kk) in enumerate(steps):
                            first, last = si == 0, si == len(steps) - 1
                            for tsub in range(C // P):
                                tsl = slice(tsub * P, (tsub + 1) * P)
                                for oc in range(OS // NF):
                                    osl = slice(oc * NF, (oc + 1) * NF)
                                    po = pos[tsub, oc]
                                    if kind == "f8":
                                        nc.tensor.matmul(
                                            po[:],
                                            xb8[:, 2 * kk : 2 * kk + 2, tsl],
                                            wq8[:, 2 * kk : 2 * kk + 2, osl],
                                            start=first, stop=last,
                                            perf_mode=(
                                                mybir.MatmulPerfMode.DoubleRow
                                            ),
                                        )
                                    else:
                                        nc.tensor.matmul(
                                            po[:],
                                            xb16[:, kk, tsl],
                                            wq16[:, kk, osl],
                                            start=first, stop=last,
                                        )
                    else:
                        for po in pos.values():
                            nc.vector.memset(po[:], 0.0)
                    for tsub in range(C // P):
                        t0 = tch * C + tsub * P
                        for oc in range(OS // NF):
                            osl = slice(oc * NF, (oc + 1) * NF)
                            po = pos[tsub, oc]
                            ob = opool.tile([P, NF], F32, name="ob", tag="ob")
                            if "scale" in SKIP:
                                nc.vector.tensor_copy(ob[:], po[:])
                            elif SCALE_ENG == "vector":
                                nc.vector.tensor_scalar_mul(
                                    ob[:], po[:], mean_t[:]
                                )
                            else:
                                nc.scalar.mul(ob[:], po[:], mean_t[:])
                            if "outdma" not in SKIP:
                                nc.sync.dma_start(out[t0 : t0 + P, osl], ob[:])
                    continue
                for oc in range(OS // NF):
                    osl = slice(oc * NF, (oc + 1) * NF)
                    for tsub in range(C // P):
                        t0 = tch * C + tsub * P
                        tsl = slice(tsub * P, (tsub + 1) * P)
                        po = psum_pool.tile([P, NF], F32, name="po", tag="po")
                        if "mm" not in SKIP:
                            i = 0
                            for kp in range(F8 // 2):
                                # fp8 DoubleRow: 2 k-tiles per instruction
                                nc.tensor.matmul(
                                    po[:],
                                    xb8[:, 2 * kp : 2 * kp + 2, tsl],
                                    wq8[:, 2 * kp : 2 * kp + 2, osl],
                                    start=(i == 0),
                                    stop=(i == n_mm - 1),
                                    perf_mode=mybir.MatmulPerfMode.DoubleRow,
                                )
                                i += 1
                            for k in range(KT16):
                                nc.tensor.matmul(
                                    po[:],
                                    xb16[:, k, tsl],
                                    wq16[:, k, osl],
                                    start=(i == 0),
                                    stop=(i == n_mm - 1),
                                )
                                i += 1
                        else:
                            nc.vector.memset(po[:], 0.0)
                        ob = opool.tile([P, NF], F32, name="ob", tag="ob")
                        if "scale" in SKIP:
                            nc.vector.tensor_copy(ob[:], po[:])
                        elif SCALE_ENG == "vector":
                            nc.vector.tensor_scalar_mul(ob[:], po[:], mean_t[:])
                        else:
                            # out = psum * mean(|W|), on the scalar engine
                            nc.scalar.mul(ob[:], po[:], mean_t[:])
                        if "outdma" not in SKIP:
                            nc.sync.dma_start(out[t0 : t0 + P, osl], ob[:])


def kernel(x, weight):
    global LAST_RESULTS
    x = np.asarray(x, dtype=np.float32)
    weight = np.asarray(weight, dtype=np.float32)
    assert x.shape == (B, S, IN_F), x.shape
    assert weight.shape == (OUT_F, IN_F), weight.shape

    xT = np.ascontiguousarray(x.reshape(T_FULL, K).T)
    in_maps = []
    for c in range(N_CORES):
        wTc = np.ascontiguousarray(weight[c * OS : (c + 1) * OS, :].T)
        in_maps.append({"xT": xT, "wT": wTc})

    cores = list(range(N_CORES))
    if CC_KIND == "2launch":
        # launch 1: per-core partial |w| sums (all math on device)
        if "a" not in _PROGRAMS:
            _PROGRAMS["a"] = _build_phase_a()
        res_a = run_bass_kernel_spmd(_PROGRAMS["a"], in_maps, cores)
        gs = np.concatenate(  # pure data movement, no host math
            [res_a.results[c]["asum"] for c in range(N_CORES)], axis=1
        )
        for m in in_maps:
            m["gsums"] = gs
    if "main" not in _PROGRAMS:
        _PROGRAMS["main"] = _build_program()
    res = run_bass_kernel_spmd(_PROGRAMS["main"], in_maps, cores)
    LAST_RESULTS = res
    outs = [res.results[c]["out"] for c in range(N_CORES)]
    return np.concatenate(outs, axis=1).reshape(B, S, OUT_F)



# revision 17
# speedup vs baseline: 1.2876x; 1.2876x over previous
"""BitLinear (ternary-quantized linear) Trainium2 kernel.

out = (x @ ternary_quantize(W).T) * mean(|W|),  alpha = 0.7

Sharding: tensor-parallel over out_features (8192 -> 8 x 1024). Every core
gets the full activation x (pre-transposed on host to [K, T] layout so all
device DMAs are contiguous) plus its own weight shard, also pre-transposed
to [K, O_shard].

weight_scale = mean(|W|) needs a global sum of |w| across the 8 shards. A
[128,1] device AllReduce works but costs ~150us per execution in this
environment (vs the ~10us documented floor), so the default is two
launches: launch 1 computes each core's partial |w| sums on device;
the host concatenates the 8 [128,1] vectors (pure data movement, no host
math) and feeds them to every core in launch 2, which sums them on device.
This also removes the weight-scan from launch 2's critical path: the
threshold is available immediately, so quantization pipelines with the
weight DMA and the tensor engine starts ~20us into the kernel.

Device kernel per core in the main launch (SPMD, identical program):
  phase 0: global mean / 0.7*mean threshold replicated on 128 partitions
           (from the gsums input; AllReduce fallback via KERNEL_CC=ar)
  phase 1: ternary-quantize the fp32 shard into {-1, 0, +1} (exact):
           k-tiles [0, F8) into fp8-e4m3 wq8, the rest into fp16 wq16;
           k-major so phase 2's first k-step is ready early, pipelined
           with the per-k-tile weight DMAs
  phase 2: tiled matmul. Per 512-token chunk, all 8 (tsub, o-half) PSUM
           banks accumulate concurrently with the k-step loop OUTERMOST:
           fp8 k-pairs via DoubleRow matmuls ([128,2,128]x[128,2,512],
           2 k-tiles per instruction), then fp16 k-tiles. The k-outer
           order means each stationary x tile is loaded once per k-step
           and reused across both o-halves, which keeps the 256-column
           DoubleRow LDWEIGHTS off the critical path (measured 1.6x vs
           the psum-group-inner-k order). x is cast fp32->fp8/fp16
           in-flight by the SWDGE DMA. PSUM is scaled by mean(|W|) on
           the scalar engine during the PSUM->SBUF copy, then DMA'd out.

Precision: weights {-1,0,+1} are exact in both fp8-e4m3 and fp16; all
error comes from quantizing x. fp16 gives rel ~1.5e-4; full fp8 gives
1.93e-2 (vs the 2e-2 gate). F8=12 of 16 k-tiles in fp8 measures
1.6785e-2, reproduced bit-identically by the CPU model (the SWDGE cast
is round-to-nearest ml_dtypes.float8_e4m3), with a 400-trial Monte
Carlo over threshold-summation-order perturbations staying under
1.88e-2.
"""

import numpy as np

import concourse.mybir as mybir
import concourse.tile as tile
from concourse import bacc, bass_isa
from concourse.bass_utils import run_bass_kernel_spmd

N_CORES = 8
B, S, IN_F, OUT_F = 4, 2048, 2048, 8192
T_FULL = B * S              # 8192 tokens
K = IN_F                    # contraction dim
OS = OUT_F // N_CORES       # 1024 out-features per core
P = 128
KT = K // P                 # 16 k-tiles
ALPHA = 0.7
N_TOTAL = float(OUT_F * K)  # 2**24, so 1/N_TOTAL is exact in fp32

C = 512                     # token chunk per x DMA
NF = 512                    # matmul moving free dim (one PSUM bank fp32)

LAST_RESULTS = None         # test harness peeks at exec_time_ns here
_PROGRAMS = {}              # compiled program cache across kernel() calls

import os as _os
SKIP = set(filter(None, _os.environ.get("KERNEL_SKIP", "").split(",")))
XDT_NAME = _os.environ.get("KERNEL_XDT", "float16")  # matmul operand dtype
SCALE_ENG = _os.environ.get("KERNEL_SCALE_ENG", "scalar")  # psum-scale engine
CC_KIND = _os.environ.get("KERNEL_CC", "2launch")  # ar | ag | 2launch
# Number of k-tiles (out of KT=16) computed in fp8-e4m3 via DoubleRow
# matmuls (2 k-tiles per instruction, ~1.44x tensor-engine throughput).
# x-quantization error grows as sqrt(F8/16) * 1.9e-2; the rest of the
# k-tiles stay fp16. Must be even.
F8 = int(_os.environ.get("KERNEL_F8", "12"))
assert F8 % 2 == 0 and 0 <= F8 <= KT
# Matmul loop order. "ocmajor": psum-group inner k (stationary x tile
# changes every matmul -> DoubleRow pays its 256-col LDWEIGHTS each MM).
# "kouter": k-step outer over all 8 psum banks of a chunk, so each
# stationary x tile is reused for both o-halves (half the LDWEIGHTS).
ORDER = _os.environ.get("KERNEL_ORDER", "kouter")
# DRAM dtype of the main launch's output. float16 halves the 32 MiB/core
# output write; values are upcast to fp32 on the host (exact). Adds
# ~2e-4 of output rounding on top of the x-quantization error.
ODT_NAME = _os.environ.get("KERNEL_ODT", "float16")


def _build_program(t_tokens=T_FULL, loops=1, n_cores=N_CORES, barrier=False):
    F32 = mybir.dt.float32
    AX = mybir.AxisListType.X
    Alu = mybir.AluOpType

    assert t_tokens % C == 0 and C % P == 0 and OS % NF == 0

    nc = bacc.Bacc(
        "TRN2", target_bir_lowering=False, debug=False, num_devices=n_cores
    )
    xT = nc.dram_tensor("xT", [K, t_tokens], F32, kind="ExternalInput").ap()
    wT = nc.dram_tensor("wT", [K, OS], F32, kind="ExternalInput").ap()
    gsums = None
    if CC_KIND == "2launch":
        gsums = nc.dram_tensor(
            "gsums", [P, N_CORES], F32, kind="ExternalInput"
        ).ap()
    ODT = getattr(mybir.dt, ODT_NAME)
    out = nc.dram_tensor("out", [t_tokens, OS], ODT, kind="ExternalOutput").ap()

    with tile.TileContext(nc) as tc:
        for _loop in range(loops):
            if barrier and _loop > 0:
                tc.strict_bb_all_engine_barrier()
            _build_body(tc, nc, xT, wT, out, t_tokens, n_cores, gsums)

    nc.compile()
    return nc


def _build_phase_a(loops=1, barrier=False):
    """Tiny first launch: per-core sum of |w shard| -> [128, 1] output."""
    F32 = mybir.dt.float32
    AX = mybir.AxisListType.X
    Alu = mybir.AluOpType
    nc = bacc.Bacc(
        "TRN2", target_bir_lowering=False, debug=False, num_devices=N_CORES
    )
    wT = nc.dram_tensor("wT", [K, OS], F32, kind="ExternalInput").ap()
    asum_out = nc.dram_tensor("asum", [P, 1], F32, kind="ExternalOutput").ap()
    with tile.TileContext(nc) as tc:
        for _loop in range(loops):
            if barrier and _loop > 0:
                tc.strict_bb_all_engine_barrier()
            with (
                tc.tile_pool(name="wpa", bufs=3) as wpa,
                tc.tile_pool(name="cpa", bufs=1) as cpa,
            ):
                # 8 x 1MB DMAs (2 k-tiles each): above the DMA batching knee,
                # reduction still pipelines with the loads
                wT_t = wT.rearrange("(n p) o -> p n o", p=P)
                KG = 2
                asum = cpa.tile([P, KT // KG], F32)
                for g in range(KT // KG):
                    wt = wpa.tile([P, KG, OS], F32, name="wt", tag="wt")
                    nc.sync.dma_start(wt[:], wT_t[:, g * KG : (g + 1) * KG, :])
                    nc.vector.tensor_reduce(
                        asum[:, g : g + 1], wt[:],
                        axis=mybir.AxisListType.XY, op=Alu.add,
                        apply_absolute_value=True,
                    )
                asum1 = cpa.tile([P, 1], F32)
                nc.vector.tensor_reduce(asum1[:], asum[:], axis=AX, op=Alu.add)
                nc.sync.dma_start(asum_out[:], asum1[:])
    nc.compile()
    return nc


def _build_phase_a_raw(loops=1):
    """Raw-bass phase A: no TileContext, so no ~10-15us exit butterfly.

    8 HWDGE DMAs into distinct SBUF regions (no reuse -> no WAR hazards),
    DVE abs-reduces pipelined behind them via one counting semaphore.
    """
    F32 = mybir.dt.float32
    Alu = mybir.AluOpType
    XY = mybir.AxisListType.XY
    AX = mybir.AxisListType.X
    KG = 2
    NG = KT // KG  # 8 groups
    nc = bacc.Bacc(
        "TRN2", target_bir_lowering=False, debug=False, num_devices=N_CORES
    )
    wT = nc.dram_tensor("wT", [K, OS], F32, kind="ExternalInput").ap()
    asum_out = nc.dram_tensor("asum", [P, 1], F32, kind="ExternalOutput").ap()
    wT_t = wT.rearrange("(n p) o -> p n o", p=P)

    import contextlib

    with contextlib.ExitStack() as ctx:
        wt = ctx.enter_context(nc.sbuf_tensor("wt", [P, KT, OS], F32))
        asum_sb = ctx.enter_context(nc.sbuf_tensor("asum_sb", [P, NG], F32))
        asum1_sb = ctx.enter_context(nc.sbuf_tensor("asum1_sb", [P, 1], F32))
        # one completion sem per DMA group: HWDGE DMAs complete out of
        # order, so a single counting sem cannot gate per-group reduces
        gsem = [
            ctx.enter_context(nc.semaphore(f"dma_sem{g}")) for g in range(NG)
        ]
        osem = ctx.enter_context(nc.semaphore("out_sem"))
        vec_sem = ctx.enter_context(nc.semaphore("vec_sem"))
        block = ctx.enter_context(nc.Block())

        @block.sync
        def _(sync):
            for it in range(loops):
                for g in range(NG):
                    sync.dma_start(
                        wt.ap()[:, g * KG : (g + 1) * KG, :],
                        wT_t[:, g * KG : (g + 1) * KG, :],
                    ).then_inc(gsem[g], 16)
                sync.wait_ge(vec_sem, it + 1)
                sync.dma_start(asum_out[:], asum1_sb.ap()[:]).then_inc(
                    osem, 16
                )
                # last byte of the output must land before the stream ends
                sync.wait_ge(osem, (it + 1) * 16)

        @block.vector
        def _(vector):
            for it in range(loops):
                for g in range(NG):
                    vector.wait_ge(gsem[g], (it + 1) * 16)
                    vector.tensor_reduce(
                        asum_sb.ap()[:, g : g + 1],
                        wt.ap()[:, g * KG : (g + 1) * KG, :],
                        axis=XY, op=Alu.add, apply_absolute_value=True,
                    )
                vector.tensor_reduce(
                    asum1_sb.ap()[:], asum_sb.ap()[:], axis=AX, op=Alu.add
                ).then_inc(vec_sem, 1)

    nc.compile()
    return nc


def _build_body(tc, nc, xT, wT, out, t_tokens, n_cores, gsums=None):
    F32 = mybir.dt.float32
    XDT = getattr(mybir.dt, XDT_NAME)
    ODT = getattr(mybir.dt, ODT_NAME)
    AX = mybir.AxisListType.X
    Alu = mybir.AluOpType
    if True:
        with (  # noqa: SIM117

            tc.tile_pool(name="wpool", bufs=1) as wpool,
            tc.tile_pool(name="cpool", bufs=1) as cpool,
            tc.tile_pool(name="dram", bufs=1, space="DRAM") as dram,
            tc.tile_pool(name="xpool", bufs=2) as xpool,
            tc.tile_pool(name="opool", bufs=6) as opool,
            tc.tile_pool(name="psum", bufs=6, space="PSUM") as psum_pool,
        ):
            # ---- phase 0: weight shard load + global mean(|W|) ----
            # per-(o-half, k-tile) DMAs in quantization order, so the first
            # o-half's quant (which gates the first matmuls) only waits for
            # half the weight bytes
            wf = wpool.tile([P, KT, OS], F32)
            wT_t = wT.rearrange("(n p) o -> p n o", p=P)
            if ORDER == "kouter":
                wdma_iter = [
                    (oc, k) for k in range(KT) for oc in range(OS // NF)
                ]
            else:
                wdma_iter = [
                    (oc, k) for oc in range(OS // NF) for k in range(KT)
                ]
            for oc, k in wdma_iter:
                osl = slice(oc * NF, (oc + 1) * NF)
                nc.sync.dma_start(wf[:, k, osl], wT_t[:, k, osl])
            if not (CC_KIND == "2launch" and gsums is not None):
                # local |w| sums feed the collective (non-2launch modes)
                asum = cpool.tile([P, KT], F32)
                for k in range(KT):
                    nc.vector.tensor_reduce(
                        asum[:, k : k + 1], wf[:, k, :], axis=AX, op=Alu.add,
                        apply_absolute_value=True,
                    )
                asum1 = cpool.tile([P, 1], F32)
                nc.vector.tensor_reduce(asum1[:], asum[:], axis=AX, op=Alu.add)

            if CC_KIND == "2launch" and gsums is not None:
                # partial |w| sums of all 8 cores arrive as an input
                gsum8 = cpool.tile([P, N_CORES], F32)
                nc.sync.dma_start(gsum8[:], gsums[:])
                gsum = cpool.tile([P, 1], F32)
                nc.vector.tensor_reduce(gsum[:], gsum8[:], axis=AX, op=Alu.add)
            elif n_cores > 1 and "ar" not in SKIP:
                if CC_KIND == "ag":
                    cc_in = dram.tile([P, 1], F32)
                    cc_out = dram.tile([n_cores * P, 1], F32)
                    nc.sync.dma_start(cc_in[:], asum1[:])
                    nc.gpsimd.collective_compute(
                        "AllGather", Alu.bypass,
                        replica_groups=[list(range(n_cores))],
                        ins=[cc_in.opt()], outs=[cc_out.opt()],
                    )
                    gsum8 = cpool.tile([P, n_cores], F32)
                    nc.sync.dma_start(
                        gsum8[:], cc_out.rearrange("(r p) o -> p (r o)", p=P)
                    )
                    gsum = cpool.tile([P, 1], F32)
                    nc.vector.tensor_reduce(
                        gsum[:], gsum8[:], axis=AX, op=Alu.add
                    )
                else:
                    cc_in = dram.tile([P, 1], F32)
                    cc_out = dram.tile([P, 1], F32)
                    nc.sync.dma_start(cc_in[:], asum1[:])
                    nc.gpsimd.collective_compute(
                        "AllReduce", Alu.add,
                        replica_groups=[list(range(n_cores))],
                        ins=[cc_in.opt()], outs=[cc_out.opt()],
                    )
                    gsum = cpool.tile([P, 1], F32)
                    nc.sync.dma_start(gsum[:], cc_out[:])
            else:
                gsum = asum1  # single-core (TimelineSim) variant

            tot = cpool.tile([P, 1], F32)
            if "par" not in SKIP:
                nc.gpsimd.partition_all_reduce(
                    tot[:], gsum[:], channels=P, reduce_op=bass_isa.ReduceOp.add
                )
            else:
                nc.vector.tensor_copy(tot[:], gsum[:])
            # mean = tot * 2**-24 (exact); thr = 0.7 * mean; both replicated
            mean_t = cpool.tile([P, 1], F32)
            nc.vector.tensor_scalar_mul(mean_t[:], tot[:], 1.0 / N_TOTAL)
            thr_t = cpool.tile([P, 1], F32)
            nc.vector.tensor_scalar_mul(thr_t[:], mean_t[:], ALPHA)
            nthr_t = cpool.tile([P, 1], F32)
            nc.vector.tensor_scalar_mul(nthr_t[:], thr_t[:], -1.0)

            # ---- phase 1: ternary quantize -> wq (exact {-1,0,+1}) ----
            # k-tiles [0, F8) go to wq8 (fp8-e4m3, consumed by DoubleRow
            # matmuls), the rest to wq16 (fp16). oc-major so the first
            # o-half is ready in half the quant time; phase 2's first chunk
            # consumes o-half 0 first.
            F8DT = mybir.dt.float8e4
            KT16 = KT - F8
            wq8 = wpool.tile([P, F8, OS], F8DT, name="wq8") if F8 else None
            wq16 = (
                wpool.tile([P, KT16, OS], XDT, name="wq16") if KT16 else None
            )

            def _wq_slot(k, osl):
                if k < F8:
                    return wq8[:, k, osl]
                return wq16[:, k - F8, osl]

            # quant iteration order matches what phase 2 consumes first:
            # ocmajor's first psum group needs all k of o-half 0; kouter's
            # first k-step needs k-tiles 0..1 across the full o range.
            if ORDER == "kouter":
                quant_iter = [
                    (oc, k) for k in range(KT) for oc in range(OS // NF)
                ]
            else:
                quant_iter = [
                    (oc, k) for oc in range(OS // NF) for k in range(KT)
                ]
            if "quant" in SKIP:
                if F8:
                    nc.vector.memset(wq8[:], 1.0)
                if KT16:
                    nc.vector.memset(wq16[:], 1.0)
            else:
                for oc, k in quant_iter:
                    osl = slice(oc * NF, (oc + 1) * NF)
                    neg = wpool.tile([P, NF], XDT, tag="negtmp")
                    # neg = (w <= -thr) in {0,1}
                    nc.vector.tensor_scalar(
                        neg[:], wf[:, k, osl], nthr_t[:], None, op0=Alu.is_le
                    )
                    # wq = (w >= thr) - neg  in {-1, 0, 1}
                    nc.vector.scalar_tensor_tensor(
                        _wq_slot(k, osl), wf[:, k, osl], thr_t[:], neg[:],
                        op0=Alu.is_ge, op1=Alu.subtract,
                    )

            # ---- phase 2: matmul sweep over tokens (oc-major per chunk so
            # the first chunk only waits on the o-half-0 quantization) ----
            xT_t = xT.rearrange("(n p) t -> p n t", p=P)
            n_chunks = t_tokens // C
            n_mm = F8 // 2 + KT16
            for tch in range(n_chunks):
                tsl_d = slice(tch * C, (tch + 1) * C)
                xb8 = xb16 = None
                if F8:
                    xb8 = xpool.tile([P, F8, C], F8DT, name="xb8", tag="xb8")
                    # SWDGE DMA with in-flight fp32 -> fp8 cast
                    nc.gpsimd.dma_start(xb8[:], xT_t[:, :F8, tsl_d])
                if KT16:
                    xb16 = xpool.tile([P, KT16, C], XDT, name="xb16", tag="xb16")
                    # SWDGE DMA with in-flight fp32 -> fp16 cast
                    nc.gpsimd.dma_start(xb16[:], xT_t[:, F8:, tsl_d])
                if ORDER == "kouter":
                    # k-step outer: all 8 (tsub, oc) psum banks accumulate
                    # concurrently, so each stationary x tile is loaded once
                    # per k-step and reused for both o-halves.
                    pos = {}
                    for tsub in range(C // P):
                        for oc in range(OS // NF):
                            pos[tsub, oc] = psum_pool.tile(
                                [P, NF], F32,
                                name="po", tag=f"po{tsub}_{oc}", bufs=1,
                            )
                    if "mm" not in SKIP:
                        steps = [("f8", kp) for kp in range(F8 // 2)]
                        steps += [("f16", k) for k in range(KT16)]
                        for si, (kind, kk) in enumerate(steps):
                            first, last = si == 0, si == len(steps) - 1
                            for tsub in range(C // P):
                                tsl = slice(tsub * P, (tsub + 1) * P)
                                for oc in range(OS // NF):
                                    osl = slice(oc * NF, (oc + 1) * NF)
                                    po = pos[tsub, oc]
                                    if kind == "f8":
                                        nc.tensor.matmul(
                                            po[:],
                                            xb8[:, 2 * kk : 2 * kk + 2, tsl],
                                            wq8[:, 2 * kk : 2 * kk + 2, osl],
                                            start=first, stop=last,
                                            perf_mode=(
                                                mybir.MatmulPerfMode.DoubleRow
                                            ),
                                        )
                                    else:
                                        nc.tensor.matmul(
                                            po[:],
                                            xb16[:, kk, tsl],
                                            wq16[:, kk, osl],
                                            start=first, stop=last,
                                        )
                    else:
                        for po in pos.values():
                            nc.vector.memset(po[:], 0.0)
                    for tsub in range(C // P):
                        t0 = tch * C + tsub * P
                        # one full-width ob per tsub so the out DMA writes
                        # whole 2 KB rows even with a 16-bit out dtype
                        ob = opool.tile([P, OS], ODT, name="ob", tag="ob")
                        for oc in range(OS // NF):
                            osl = slice(oc * NF, (oc + 1) * NF)
                            po = pos[tsub, oc]
                            if "scale" in SKIP:
                                nc.vector.tensor_copy(ob[:, osl], po[:])
                            elif SCALE_ENG == "vector":
                                nc.vector.tensor_scalar_mul(
                                    ob[:, osl], po[:], mean_t[:]
                                )
                            else:
                                nc.scalar.mul(ob[:, osl], po[:], mean_t[:])
                        if "outdma" not in SKIP:
                            nc.sync.dma_start(out[t0 : t0 + P, :], ob[:])
                    continue
                for oc in range(OS // NF):
                    osl = slice(oc * NF, (oc + 1) * NF)
                    for tsub in range(C // P):
                        t0 = tch * C + tsub * P
                        tsl = slice(tsub * P, (tsub + 1) * P)
                        po = psum_pool.tile([P, NF], F32, name="po", tag="po")
                        if "mm" not in SKIP:
                            i = 0
                            for kp in range(F8 // 2):
                                # fp8 DoubleRow: 2 k-tiles per instruction
                                nc.tensor.matmul(
                                    po[:],
                                    xb8[:, 2 * kp : 2 * kp + 2, tsl],
                                    wq8[:, 2 * kp : 2 * kp + 2, osl],
                                    start=(i == 0),
                                    stop=(i == n_mm - 1),
                                    perf_mode=mybir.MatmulPerfMode.DoubleRow,
                                )
                                i += 1
                            for k in range(KT16):
                                nc.tensor.matmul(
                                    po[:],
                                    xb16[:, k, tsl],
                                    wq16[:, k, osl],
                                    start=(i == 0),
                                    stop=(i == n_mm - 1),
                                )
                                i += 1
                        else:
                            nc.vector.memset(po[:], 0.0)
                        ob = opool.tile([P, NF], ODT, name="ob", tag="ob")
                        if "scale" in SKIP:
                            nc.vector.tensor_copy(ob[:], po[:])
                        elif SCALE_ENG == "vector":
                            nc.vector.tensor_scalar_mul(ob[:], po[:], mean_t[:])
                        else:
                            # out = psum * mean(|W|), on the scalar engine
                            nc.scalar.mul(ob[:], po[:], mean_t[:])
                        if "outdma" not in SKIP:
                            nc.sync.dma_start(out[t0 : t0 + P, osl], ob[:])


def kernel(x, weight):
    global LAST_RESULTS
    x = np.asarray(x, dtype=np.float32)
    weight = np.asarray(weight, dtype=np.float32)
    assert x.shape == (B, S, IN_F), x.shape
    assert weight.shape == (OUT_F, IN_F), weight.shape

    xT = np.ascontiguousarray(x.reshape(T_FULL, K).T)
    in_maps = []
    for c in range(N_CORES):
        wTc = np.ascontiguousarray(weight[c * OS : (c + 1) * OS, :].T)
        in_maps.append({"xT": xT, "wT": wTc})

    cores = list(range(N_CORES))
    if CC_KIND == "2launch":
        # launch 1: per-core partial |w| sums (all math on device)
        if "a" not in _PROGRAMS:
            _PROGRAMS["a"] = _build_phase_a()
        res_a = run_bass_kernel_spmd(_PROGRAMS["a"], in_maps, cores)
        gs = np.concatenate(  # pure data movement, no host math
            [res_a.results[c]["asum"] for c in range(N_CORES)], axis=1
        )
        for m in in_maps:
            m["gsums"] = gs
    if "main" not in _PROGRAMS:
        _PROGRAMS["main"] = _build_program()
    res = run_bass_kernel_spmd(_PROGRAMS["main"], in_maps, cores)
    LAST_RESULTS = res
    outs = [res.results[c]["out"] for c in range(N_CORES)]
    full = np.concatenate(outs, axis=1)
    if full.dtype != np.float32:
        full = full.astype(np.float32)  # exact fp16 -> fp32 upcast
    return full.reshape(B, S, OUT_F)

